# revision 7
# baseline (speedup 1.0000x reference)
"""DeepseekMoE (moe_routing) Trainium2 kernel.

Strategy (8 NeuronCores, single SPMD program, DMA-roofline oriented):
  - Routing (grouped top-k) runs on host in numpy.
  - Slot A: the four heaviest experts, each TENSOR-parallel over 2 cores
    (intermediate dim 1408 -> 2 x 704, zero-padded to 768). Each core
    streams only half the expert's weights; the two partial outputs are
    summed on host at combine time. Capacity = max top-4 token count.
  - Slot B: the next six experts, token-SPLIT first-fit into 8
    instances of a small capacity (the per-instance weight stream is
    the full expert, so only 2 weight streams per core total).
  - Tiny tail experts (~2.6% of tokens) are computed on host, like the
    routing already is.
  - Shared expert MLP is tensor-parallel over the 8 cores along the
    intermediate dim (2816 -> 8 x 352, zero-padded to 8 x 384); its
    weights are SBUF-resident (no per-iteration DMA).
  - GEMM1 pairs of all slots are interleaved (even DMA demand); shared
    GEMM2 (zero DMA) runs while the w2 streams fill the queue.
  - All matmul operands bf16, accumulation f32, outputs bf16.
"""

import numpy as np
import ml_dtypes

import concourse.mybir as mybir
import concourse.tile as tile
from concourse import bacc
from concourse.bass_utils import run_bass_kernel_spmd

BF16 = ml_dtypes.bfloat16
F32 = np.float32

# Problem shapes (fixed by the spec).
T, H, E, I = 1024, 2048, 16, 1408
I2 = 2 * I                      # 2816 (w13 rows per expert)
IS = 2 * I                      # shared intermediate (n_shared=2 -> 2816)
SSH = 384                       # per-core shared shard (2816 padded to 3072 = 8*384)
IH = 768                        # per-core A-slot half intermediate (704 padded)
TOP_K, N_GROUP, TOPK_GROUP = 4, 4, 2
ROUTED_SCALE = 2.5
N_CORES = 8
P = 128
KH = H // P                     # 16 K-subtiles over H
KI = I // P                     # 11 K-subtiles over I (full expert GEMM2)
MH = H // P                     # 16 M-panels over H
NPAIR = I // P                  # 11 (g,u) SwiGLU pairs per full expert
NPAIR_TP = IH // P              # 6 pairs per TP-half expert
KS = SSH // P                   # 3 K-subtiles over shared shard


def _sigmoid(x):
    return 1.0 / (1.0 + np.exp(-x))


def _route(x, gate_weight, gate_bias):
    """Numpy port of reference._grouped_topk (float64 internally)."""
    logits = x.astype(np.float64) @ gate_weight.astype(np.float64).T
    scores = _sigmoid(logits)
    choice = scores + gate_bias.astype(np.float64)[None, :]
    g = choice.reshape(T, N_GROUP, E // N_GROUP)
    top2sum = np.sort(g, axis=-1)[..., -2:].sum(-1)          # [T, NG]
    gidx = np.argsort(-top2sum, axis=-1, kind="stable")[:, :TOPK_GROUP]
    gmask = np.zeros((T, N_GROUP), bool)
    gmask[np.arange(T)[:, None], gidx] = True
    emask = np.repeat(gmask, E // N_GROUP, axis=1)           # [T, E]
    masked = np.where(emask, choice, -np.inf)
    topk_ids = np.argsort(-masked, axis=-1, kind="stable")[:, :TOP_K]
    topk_w = np.take_along_axis(scores, topk_ids, axis=1)
    topk_w = topk_w / topk_w.sum(-1, keepdims=True) * ROUTED_SCALE
    return topk_ids.astype(np.int32), topk_w


def _silu_mul_np(gu):
    g, u = gu[:, :gu.shape[1] // 2], gu[:, gu.shape[1] // 2:]
    return (g / (1.0 + np.exp(-g))) * u


def _pack_lhs_panels(w, n_m, n_k):
    """[n_m*128, n_k*128] (indexed [M, K]) -> [n_m, 128, n_k, 128] panels
    where panel[m][p, k, j] = w[128*m + j, 128*k + p], i.e. each panel
    slice [:, k, :] is the lhsT chunk [K-sub=128, M-sub=128]."""
    a = w.reshape(n_m, P, n_k, P)          # [m, j, k, p]
    return np.ascontiguousarray(a.transpose(0, 3, 2, 1))


def _pack_pairs(gu, npair):
    """[2*npair*128, H] g/u-stacked weight -> [npair, P, KH, 2*P] where
    pair pr holds the g panel (m=pr) in cols [0:128] and the u panel
    (m=pr+npair) in cols [128:256] -- one contiguous DMA per pair."""
    panels = _pack_lhs_panels(gu, 2 * npair, KH)
    return np.ascontiguousarray(
        np.concatenate([panels[:npair], panels[npair:]], axis=-1))


def _pack_rhs(xcols):
    """[C, H] token-major rows -> [128, KH, C] rhs layout:
    out[p, k, c] = xcols[c, 128*k + p]."""
    a = xcols.reshape(-1, KH, P)           # [c, k, p]
    return np.ascontiguousarray(a.transpose(2, 1, 0))


def _nchunks(c):
    out = []
    o = 0
    while o < c:
        n = min(512, c - o)
        out.append((o, n))
        o += n
    return out


# Per-(CB, CS) slot geometry: (npair_s, nk2_s) for each slot. Populated
# by _prepare before _build_program is called; defaults to full experts.
_SLOT_SPECS = {}


def _build_program(CB, CS, reps=1):
    """One SPMD Tile program shared by all 8 cores. CB/CS: slot
    capacities (CS may be 0 to drop the second slot). reps>1 wraps the
    compute in a hardware loop (timing amplification only)."""
    nc = bacc.Bacc(None, target_bir_lowering=False)
    bf = mybir.dt.bfloat16
    f32 = mybir.dt.float32

    slot_caps = [c for c in (CB, CS) if c > 0]
    ns = len(slot_caps)
    npairs, nk2s = _SLOT_SPECS.get((CB, CS), ((NPAIR,) * ns, (KI,) * ns))
    nk_all = sum(nk2s)

    # --- I/O ----------------------------------------------------------
    w13q = [nc.dram_tensor(f"w13q{s}", [npairs[s], P, KH, 2 * P], bf,
                           kind="ExternalInput") for s in range(ns)]
    w2q = nc.dram_tensor("w2q", [MH, P, nk_all, P], bf, kind="ExternalInput")
    xgq = [nc.dram_tensor(f"xgq{s}", [P, KH, slot_caps[s]], bf, kind="ExternalInput")
           for s in range(ns)]
    wtb = [nc.dram_tensor(f"wtb{s}", [P, slot_caps[s]], f32, kind="ExternalInput")
           for s in range(ns)]
    sguq = nc.dram_tensor("sguq", [2 * KS, P, KH, P], bf, kind="ExternalInput")
    sdq = nc.dram_tensor("sdq", [MH, P, KS, P], bf, kind="ExternalInput")
    xtq = nc.dram_tensor("xtq", [P, KH, T], bf, kind="ExternalInput")

    yout = [nc.dram_tensor(f"y{s}", [MH, P, slot_caps[s]], bf, kind="ExternalOutput")
            for s in range(ns)]
    shp = nc.dram_tensor("shp", [MH, P, T], bf, kind="ExternalOutput")

    with tile.TileContext(nc) as tc:
        with (
            tc.tile_pool(name="resident", bufs=1) as res,
            tc.tile_pool(name="wpanel", bufs=4) as wpool,
            tc.tile_pool(name="w2panel", bufs=3) as w2pool,
            tc.tile_pool(name="hbuf", bufs=1) as hpool,
            tc.tile_pool(name="silu", bufs=3) as spool,
            tc.tile_pool(name="outbuf", bufs=3) as opool,
            tc.tile_pool(name="psum", bufs=8, space="PSUM") as psum1,
        ):
            # Resident activations + shared-expert weights
            xg_t, wt_t = [], []
            for s in range(ns):
                c = slot_caps[s]
                t = res.tile([P, KH, c], bf, name=f"xg{s}_t")
                nc.sync.dma_start(t[:], xgq[s].ap()[:])
                xg_t.append(t)
                w = res.tile([P, c], f32, name=f"wt{s}_t")
                nc.sync.dma_start(w[:], wtb[s].ap()[:])
                wt_t.append(w)
            xt_t = res.tile([P, KH, T], bf)
            nc.sync.dma_start(xt_t[:], xtq.ap()[:])
            sgu_t = res.tile([P, KH, 2 * KS * P], bf)   # resident shared gate_up
            for m in range(2 * KS):
                nc.sync.dma_start(sgu_t[:, :, m * P:(m + 1) * P], sguq.ap()[m])
            sd_t = res.tile([P, KS, H], bf)             # resident shared-down
            for m in range(MH):
                nc.sync.dma_start(sd_t[:, :, m * P:(m + 1) * P], sdq.ap()[m])

            h_t = [hpool.tile([P, nk2s[s], slot_caps[s]], bf, name=f"h{s}_t",
                              tag=f"h{s}_t") for s in range(ns)]
            hs_t = hpool.tile([P, KS, T], bf)

            def swiglu(psg, psu, cap, h_out, pr):
                for ci, (o, n) in enumerate(_nchunks(cap)):
                    # silu(g) * u as sigmoid(g) * g * u (Silu itself is
                    # not implemented in CoreSim).
                    sg = spool.tile([P, 512], mybir.dt.float32, tag="sg")
                    nc.scalar.activation(
                        sg[:, :n], psg[ci][:, :n],
                        mybir.ActivationFunctionType.Sigmoid,
                    )
                    nc.vector.tensor_mul(sg[:, :n], sg[:, :n], psg[ci][:, :n])
                    nc.vector.tensor_mul(
                        h_out[:, pr, o:o + n], sg[:, :n], psu[ci][:, :n])

            def g1_pair(s, pr):
                """Routed slot s, SwiGLU pair pr: one packed [g|u] panel
                DMA, 2x16 matmuls, SwiGLU evict into h_t[s]."""
                cap = slot_caps[s]
                pan = wpool.tile([P, KH, 2 * P], bf, tag="wpanel1")
                nc.sync.dma_start(pan[:], w13q[s].ap()[pr])
                psums = []
                for half in range(2):      # 0: g, 1: u
                    ps = [psum1.tile([P, 512], mybir.dt.float32, tag="ps",
                                     name=f"ps_g1_{s}_{pr}_{half}_{ci}")
                          for ci in range(len(_nchunks(cap)))]
                    for k in range(KH):
                        for ci, (o, n) in enumerate(_nchunks(cap)):
                            nc.tensor.matmul(
                                ps[ci][:, :n],
                                lhsT=pan[:, k, half * P:(half + 1) * P],
                                rhs=xg_t[s][:, k, o:o + n],
                                start=(k == 0),
                                stop=(k == KH - 1),
                            )
                    psums.append(ps)
                swiglu(psums[0], psums[1], cap, h_t[s], pr)

            def g1_pair_shared(pr):
                psums = []
                for m in (pr, pr + KS):
                    ps = [psum1.tile([P, 512], mybir.dt.float32, tag="ps",
                                     name=f"ps_g1s_{pr}_{m}_{ci}")
                          for ci in range(len(_nchunks(T)))]
                    for k in range(KH):
                        for ci, (o, n) in enumerate(_nchunks(T)):
                            nc.tensor.matmul(
                                ps[ci][:, :n],
                                lhsT=sgu_t[:, k, m * P:(m + 1) * P],
                                rhs=xt_t[:, k, o:o + n],
                                start=(k == 0),
                                stop=(k == KH - 1),
                            )
                    psums.append(ps)
                swiglu(psums[0], psums[1], T, hs_t, pr)

            def g2_panel(m):
                """GEMM2 m-panel for all routed slots: one merged w2 DMA."""
                pan = w2pool.tile([P, nk_all, P], bf, tag="wpanel2")
                nc.sync.dma_start(pan[:], w2q.ap()[m])
                koff = 0
                for s in range(ns):
                    cap = slot_caps[s]
                    ps = [psum1.tile([P, 512], mybir.dt.float32, tag="ps",
                                     name=f"ps_g2_{m}_{s}_{ci}")
                          for ci in range(len(_nchunks(cap)))]
                    for k in range(nk2s[s]):
                        for ci, (o, n) in enumerate(_nchunks(cap)):
                            nc.tensor.matmul(
                                ps[ci][:, :n],
                                lhsT=pan[:, koff + k, :],
                                rhs=h_t[s][:, k, o:o + n],
                                start=(k == 0),
                                stop=(k == nk2s[s] - 1),
                            )
                    koff += nk2s[s]
                    ot = opool.tile([P, cap], bf, tag=f"yout{s}")
                    for ci, (o, n) in enumerate(_nchunks(cap)):
                        nc.vector.tensor_mul(
                            ot[:, o:o + n], ps[ci][:, :n], wt_t[s][:, o:o + n])
                    nc.scalar.dma_start(yout[s].ap()[m], ot[:])

            def sh_g2(m):
                ps = [psum1.tile([P, 512], mybir.dt.float32, tag="ps",
                                 name=f"ps_sh_{m}_{ci}")
                      for ci in range(len(_nchunks(T)))]
                for k in range(KS):
                    for ci, (o, n) in enumerate(_nchunks(T)):
                        nc.tensor.matmul(
                            ps[ci][:, :n],
                            lhsT=sd_t[:, k, m * P:(m + 1) * P],
                            rhs=hs_t[:, k, o:o + n],
                            start=(k == 0),
                            stop=(k == KS - 1),
                        )
                ot = opool.tile([P, T], bf, tag="shout")
                for ci, (o, n) in enumerate(_nchunks(T)):
                    nc.any.tensor_copy(ot[:, o:o + n], ps[ci][:, :n])
                nc.scalar.dma_start(shp.ap()[m], ot[:])

            def body():
                # GEMM1: interleave slot pairs so the weight-DMA stream is
                # even; the 3 shared pairs (zero DMA) fill the back half
                # where only the small slot still has pairs.
                np_max = max(npairs)
                shared_at = {np_max - 5: 0, np_max - 3: 1, np_max - 1: 2}
                for pr in range(np_max):
                    for s in range(ns):
                        if pr < npairs[s]:
                            g1_pair(s, pr)
                    j = shared_at.get(pr)
                    if j is not None:
                        g1_pair_shared(j)
                # Shared GEMM2 first: sd is resident (zero DMA), so the PE
                # chews it while the w2 panel stream fills the DMA queue.
                for m in range(MH):
                    sh_g2(m)
                for m in range(MH):
                    g2_panel(m)

            if reps == 1:
                body()
            else:
                with tc.For_i(0, reps, 1):
                    body()

    nc.compile()
    return nc


_PROGRAM_CACHE = {}


def _get_program(CB, CS):
    key = (CB, CS)
    if key not in _PROGRAM_CACHE:
        _PROGRAM_CACHE[key] = _build_program(CB, CS)
    return _PROGRAM_CACHE[key]


def _assign_slots(counts):
    """Slot-instance assignment. Returns (CA, CB, instA, instB, tail).
    instA: 8 instances (expert, half) -- top-4 experts TP-split in two.
    instB: 8 instances (expert, lo, hi) -- token-split chunks.
    tail: expert ids computed on host.
    Falls back to (None-tp) big/small token assignment for flat
    distributions."""
    order = np.argsort(-counts, kind="stable")
    rnd8 = lambda v: max(8, int(-(-v // 8)) * 8)
    tail = [int(e) for e in order[10:] if counts[e] > 0]
    tail_tokens = int(sum(counts[e] for e in tail))
    if tail_tokens <= 512:
        CA = rnd8(int(counts[order[0]]))
        instA = []
        for i in range(4):
            e = int(order[i])
            instA += [(e, 0), (e, 1)]
        mid = [int(order[i]) for i in range(4, 10) if counts[order[i]] > 0]
        CB = rnd8(max(1, int(-(-sum(int(counts[e]) for e in mid) // 8))))
        while sum(-(-int(counts[e]) // CB) for e in mid) > 8:
            CB += 8
        instB = []
        for e in mid:
            n = int(counts[e])
            lo = 0
            while lo < n:
                hi = min(lo + CB, n)
                instB.append((e, lo, hi))
                lo = hi
        while len(instB) < 8:
            instB.append((int(order[0]), 0, 0))   # dead instance
        return CA, CB, instA, instB, tail
    # Fallback: one big + one small expert per core, full experts.
    big, small = order[:N_CORES], order[N_CORES:]
    CA = rnd8(int(counts[big].max()))
    CB = rnd8(int(counts[small].max())) if counts[small].max() > 0 else 0
    instA = [(int(e), 0, int(counts[e])) for e in big]
    instB = [(int(e), 0, int(counts[e])) for e in small[::-1]]
    return CA, CB, instA, instB, None


def _pack_w13_tp(w13_e, half):
    """TP half of w13 for one expert: g rows [j*704:(j+1)*704] and the
    matching u rows, each zero-padded to 768, pair-packed."""
    gu = np.zeros((2 * IH, H), dtype=BF16)
    lo, hi = half * (I // 2), (half + 1) * (I // 2)
    gu[:hi - lo] = w13_e[lo:hi].astype(BF16)
    gu[IH:IH + hi - lo] = w13_e[I + lo:I + hi].astype(BF16)
    return _pack_pairs(gu, NPAIR_TP)


def _pack_w2_tp(w2_e, half):
    """TP half of w2: columns [j*704:(j+1)*704] zero-padded to 768."""
    w2h = np.zeros((H, IH), dtype=BF16)
    lo, hi = half * (I // 2), (half + 1) * (I // 2)
    w2h[:, :hi - lo] = w2_e[:, lo:hi].astype(BF16)
    return _pack_lhs_panels(w2h, MH, IH // P)


def _prepare(x, gate_weight, gate_bias, w13, w2, shared_gate_up, shared_down):
    """Host-side routing + packing. Returns (CB, CS, in_maps, meta)."""
    topk_ids, topk_w = _route(x, gate_weight, gate_bias)
    flat_e = topk_ids.ravel()
    flat_w = topk_w.ravel()
    flat_t = np.repeat(np.arange(T, dtype=np.int64), TOP_K)
    idx_e = [flat_t[flat_e == e] for e in range(E)]
    w_e = [flat_w[flat_e == e] for e in range(E)]
    counts = np.array([len(i) for i in idx_e])

    CB, CS, instA, instB, tail = _assign_slots(counts)
    tp_mode = tail is not None
    if tp_mode:
        _SLOT_SPECS[(CB, CS)] = ((NPAIR_TP, NPAIR), (IH // P, KI))
    else:
        ns = 2 if CS else 1
        _SLOT_SPECS[(CB, CS)] = ((NPAIR,) * ns, (KI,) * ns)
        tail = []

    xt_pack = _pack_rhs(x.astype(BF16))                 # [128, KH, T]

    # Pre-pack per-expert full panels once (B instances may share).
    packed13 = {}
    packed2 = {}

    in_maps, meta = [], []
    for c in range(N_CORES):
        insts = [instA[c]] + ([instB[c]] if CS else [])
        caps = [CB] + ([CS] if CS else [])
        im = {}
        cmeta = []
        w2panels = []
        for s, (inst, cap) in enumerate(zip(insts, caps)):
            if tp_mode and s == 0:
                e, half = inst
                idx = idx_e[e]
                im[f"w13q{s}"] = _pack_w13_tp(w13[e], half)
                w2panels.append(_pack_w2_tp(w2[e], half))
            else:
                e, lo, hi = inst
                idx = idx_e[e][lo:hi]
                if e not in packed13:
                    gu = w13[e].astype(BF16)
                    packed13[e] = _pack_pairs(gu, NPAIR)
                    packed2[e] = _pack_lhs_panels(w2[e].astype(BF16), MH, KI)
                im[f"w13q{s}"] = packed13[e]
                w2panels.append(packed2[e])
            n = len(idx)
            xg = np.zeros((cap, H), dtype=BF16)
            xg[:n] = x[idx].astype(BF16)
            im[f"xgq{s}"] = _pack_rhs(xg)
            wt = np.zeros((cap,), dtype=F32)
            wt[:n] = (w_e[inst[0]] if (tp_mode and s == 0)
                      else w_e[inst[0]][inst[1]:inst[2]]).astype(F32)
            im[f"wtb{s}"] = np.ascontiguousarray(
                np.broadcast_to(wt[None, :], (P, cap)).astype(F32))
            cmeta.append((s, inst[0], idx))
        # merged w2 stream: concat along the k-subtile axis
        im["w2q"] = np.ascontiguousarray(np.concatenate(w2panels, axis=2))
        # shared shard: rows [c*352, (c+1)*352) of gate and up, padded to 384
        sh = IS // N_CORES
        lo, hi = c * sh, (c + 1) * sh
        gsl = np.zeros((SSH, H), dtype=F32)
        usl = np.zeros((SSH, H), dtype=F32)
        gsl[:hi - lo] = shared_gate_up[lo:hi]
        usl[:hi - lo] = shared_gate_up[IS + lo:IS + hi]
        sgu_pad = np.concatenate([gsl, usl], 0).astype(BF16)   # [768, H]
        im["sguq"] = _pack_lhs_panels(sgu_pad, 2 * KS, KH)
        sd_sl = np.zeros((H, SSH), dtype=F32)
        sd_sl[:, :hi - lo] = shared_down[:, lo:hi]
        im["sdq"] = _pack_lhs_panels(sd_sl.astype(BF16), MH, KS)
        im["xtq"] = xt_pack
        in_maps.append(im)
        meta.append(cmeta)

    # Tail experts (tiny token counts): computed host-side, f32.
    tail_add = []
    for e in tail:
        idx = idx_e[e]
        if len(idx) == 0:
            continue
        xt32 = x[idx].astype(F32)
        gu = xt32 @ w13[e].astype(F32).T
        y = _silu_mul_np(gu) @ w2[e].astype(F32).T
        tail_add.append((idx, y * w_e[e].astype(F32)[:, None]))
    meta = {"slots": meta, "tail_add": tail_add}
    return CB, CS, in_maps, meta


def _combine(results, meta):
    out = np.zeros((H, T), dtype=F32)
    for c in range(N_CORES):
        out += results[c]["shp"].reshape(H, T).astype(F32)
    out = np.ascontiguousarray(out.T)                   # [T, H]
    for c in range(N_CORES):
        r = results[c]
        for (s, e, idx) in meta["slots"][c]:
            n = len(idx)
            if n:
                y = r[f"y{s}"].reshape(H, -1).astype(F32)   # [H, cap]
                out[idx] += y[:, :n].T
    for idx, y in meta["tail_add"]:
        out[idx] += y
    return out


def kernel(hidden_states, gate_weight, gate_bias, w13, w2,
           shared_gate_up, shared_down):
    x = np.asarray(hidden_states, dtype=F32)
    gate_weight = np.asarray(gate_weight, dtype=F32)
    gate_bias = np.asarray(gate_bias, dtype=F32)
    w13 = np.asarray(w13, dtype=F32)
    w2 = np.asarray(w2, dtype=F32)
    shared_gate_up = np.asarray(shared_gate_up, dtype=F32)
    shared_down = np.asarray(shared_down, dtype=F32)

    CB, CS, in_maps, meta = _prepare(
        x, gate_weight, gate_bias, w13, w2, shared_gate_up, shared_down)
    nc = _get_program(CB, CS)
    res = run_bass_kernel_spmd(nc, in_maps, core_ids=list(range(N_CORES)))
    return _combine(res.results, meta)


# revision 11
# speedup vs baseline: 1.0247x; 1.0247x over previous
"""DeepseekMoE (moe_routing) Trainium2 kernel.

Strategy (8 NeuronCores, single SPMD program, DMA-roofline oriented):
  - Routing (grouped top-k) runs on host in numpy.
  - Slot A: the four heaviest experts, each TENSOR-parallel over 2 cores
    (intermediate dim 1408 -> 2 x 704, zero-padded to 768). Each core
    streams only half the expert's weights; the two partial outputs are
    summed on host at combine time. Capacity = max top-4 token count.
  - Slot B: the next six experts, token-SPLIT first-fit into 8
    instances of a small capacity (the per-instance weight stream is
    the full expert, so only 2 weight streams per core total).
  - Tiny tail experts (~2.6% of tokens) are computed on host, like the
    routing already is.
  - Shared expert MLP is tensor-parallel over the 8 cores along the
    intermediate dim (2816 -> 8 x 352, zero-padded to 8 x 384); its
    weights are SBUF-resident (no per-iteration DMA).
  - GEMM1 pairs of all slots are interleaved (even DMA demand); shared
    GEMM2 (zero DMA) runs while the w2 streams fill the queue.
  - All matmul operands bf16, accumulation f32, outputs bf16.
"""

import numpy as np
import ml_dtypes

import concourse.mybir as mybir
import concourse.tile as tile
from concourse import bacc
from concourse.bass_utils import run_bass_kernel_spmd

BF16 = ml_dtypes.bfloat16
F32 = np.float32

# Problem shapes (fixed by the spec).
T, H, E, I = 1024, 2048, 16, 1408
I2 = 2 * I                      # 2816 (w13 rows per expert)
IS = 2 * I                      # shared intermediate (n_shared=2 -> 2816)
SSH = 384                       # per-core shared shard (2816 padded to 3072 = 8*384)
IH = 768                        # per-core A-slot half intermediate (704 padded)
TOP_K, N_GROUP, TOPK_GROUP = 4, 4, 2
ROUTED_SCALE = 2.5
N_CORES = 8
P = 128
KH = H // P                     # 16 K-subtiles over H
KI = I // P                     # 11 K-subtiles over I (full expert GEMM2)
MH = H // P                     # 16 M-panels over H
NPAIR = I // P                  # 11 (g,u) SwiGLU pairs per full expert
NPAIR_TP = IH // P              # 6 pairs per TP-half expert
KS = SSH // P                   # 3 K-subtiles over shared shard


def _sigmoid(x):
    return 1.0 / (1.0 + np.exp(-x))


def _route(x, gate_weight, gate_bias):
    """Numpy port of reference._grouped_topk (float64 internally)."""
    logits = x.astype(np.float64) @ gate_weight.astype(np.float64).T
    scores = _sigmoid(logits)
    choice = scores + gate_bias.astype(np.float64)[None, :]
    g = choice.reshape(T, N_GROUP, E // N_GROUP)
    top2sum = np.sort(g, axis=-1)[..., -2:].sum(-1)          # [T, NG]
    gidx = np.argsort(-top2sum, axis=-1, kind="stable")[:, :TOPK_GROUP]
    gmask = np.zeros((T, N_GROUP), bool)
    gmask[np.arange(T)[:, None], gidx] = True
    emask = np.repeat(gmask, E // N_GROUP, axis=1)           # [T, E]
    masked = np.where(emask, choice, -np.inf)
    topk_ids = np.argsort(-masked, axis=-1, kind="stable")[:, :TOP_K]
    topk_w = np.take_along_axis(scores, topk_ids, axis=1)
    topk_w = topk_w / topk_w.sum(-1, keepdims=True) * ROUTED_SCALE
    return topk_ids.astype(np.int32), topk_w


def _silu_mul_np(gu):
    g, u = gu[:, :gu.shape[1] // 2], gu[:, gu.shape[1] // 2:]
    return (g / (1.0 + np.exp(-g))) * u


def _pack_lhs_panels(w, n_m, n_k):
    """[n_m*128, n_k*128] (indexed [M, K]) -> [n_m, 128, n_k, 128] panels
    where panel[m][p, k, j] = w[128*m + j, 128*k + p], i.e. each panel
    slice [:, k, :] is the lhsT chunk [K-sub=128, M-sub=128]."""
    a = w.reshape(n_m, P, n_k, P)          # [m, j, k, p]
    return np.ascontiguousarray(a.transpose(0, 3, 2, 1))


def _pack_pairs(gu, npair):
    """[2*npair*128, H] g/u-stacked weight -> [npair, P, KH, 2*P] where
    pair pr holds the g panel (m=pr) in cols [0:128] and the u panel
    (m=pr+npair) in cols [128:256] -- one contiguous DMA per pair."""
    panels = _pack_lhs_panels(gu, 2 * npair, KH)
    return np.ascontiguousarray(
        np.concatenate([panels[:npair], panels[npair:]], axis=-1))


def _pack_rhs(xcols):
    """[C, H] token-major rows -> [128, KH, C] rhs layout:
    out[p, k, c] = xcols[c, 128*k + p]."""
    a = xcols.reshape(-1, KH, P)           # [c, k, p]
    return np.ascontiguousarray(a.transpose(2, 1, 0))


def _nchunks(c):
    out = []
    o = 0
    while o < c:
        n = min(512, c - o)
        out.append((o, n))
        o += n
    return out


# Per-(CB, CS) slot geometry: (npair_s, nk2_s) for each slot. Populated
# by _prepare before _build_program is called; defaults to full experts.
_SLOT_SPECS = {}


def _build_program(CB, CS, reps=1):
    """One SPMD Tile program shared by all 8 cores. CB/CS: slot
    capacities (CS may be 0 to drop the second slot). reps>1 wraps the
    compute in a hardware loop (timing amplification only)."""
    nc = bacc.Bacc(None, target_bir_lowering=False)
    bf = mybir.dt.bfloat16
    f32 = mybir.dt.float32

    slot_caps = [c for c in (CB, CS) if c > 0]
    ns = len(slot_caps)
    npairs, nk2s = _SLOT_SPECS.get((CB, CS), ((NPAIR,) * ns, (KI,) * ns))
    nk_all = sum(nk2s)

    # --- I/O ----------------------------------------------------------
    w13q = [nc.dram_tensor(f"w13q{s}", [npairs[s], P, KH, 2 * P], bf,
                           kind="ExternalInput") for s in range(ns)]
    w2q = nc.dram_tensor("w2q", [MH, P, nk_all, P], bf, kind="ExternalInput")
    xgq = [nc.dram_tensor(f"xgq{s}", [P, KH, slot_caps[s]], bf, kind="ExternalInput")
           for s in range(ns)]
    wtb = [nc.dram_tensor(f"wtb{s}", [P, slot_caps[s]], f32, kind="ExternalInput")
           for s in range(ns)]
    sguq = nc.dram_tensor("sguq", [2 * KS, P, KH, P], bf, kind="ExternalInput")
    sdq = nc.dram_tensor("sdq", [MH, P, KS, P], bf, kind="ExternalInput")
    xtq = nc.dram_tensor("xtq", [P, KH, T], bf, kind="ExternalInput")

    yout = [nc.dram_tensor(f"y{s}", [MH, P, slot_caps[s]], bf, kind="ExternalOutput")
            for s in range(ns)]
    shp = nc.dram_tensor("shp", [MH, P, T], bf, kind="ExternalOutput")

    with tile.TileContext(nc) as tc:
        with (
            tc.tile_pool(name="resident", bufs=1) as res,
            tc.tile_pool(name="wpanel", bufs=4) as wpool,
            tc.tile_pool(name="w2panel", bufs=3) as w2pool,
            tc.tile_pool(name="hbuf", bufs=1) as hpool,
            tc.tile_pool(name="silu", bufs=3) as spool,
            tc.tile_pool(name="outbuf", bufs=3) as opool,
            tc.tile_pool(name="psum", bufs=8, space="PSUM") as psum1,
        ):
            # Resident activations + shared-expert weights
            xg_t, wt_t = [], []
            for s in range(ns):
                c = slot_caps[s]
                t = res.tile([P, KH, c], bf, name=f"xg{s}_t")
                nc.sync.dma_start(t[:], xgq[s].ap()[:])
                xg_t.append(t)
                w = res.tile([P, c], f32, name=f"wt{s}_t")
                nc.sync.dma_start(w[:], wtb[s].ap()[:])
                wt_t.append(w)
            xt_t = res.tile([P, KH, T], bf)
            nc.sync.dma_start(xt_t[:], xtq.ap()[:])
            sgu_t = res.tile([P, KH, 2 * KS * P], bf)   # resident shared gate_up
            for m in range(2 * KS):
                nc.sync.dma_start(sgu_t[:, :, m * P:(m + 1) * P], sguq.ap()[m])
            sd_t = res.tile([P, KS, H], bf)             # resident shared-down
            for m in range(MH):
                nc.sync.dma_start(sd_t[:, :, m * P:(m + 1) * P], sdq.ap()[m])

            h_t = [hpool.tile([P, nk2s[s], slot_caps[s]], bf, name=f"h{s}_t",
                              tag=f"h{s}_t") for s in range(ns)]
            hs_t = hpool.tile([P, KS, T], bf)

            def swiglu(psg, psu, cap, h_out, pr):
                for ci, (o, n) in enumerate(_nchunks(cap)):
                    # silu(g) * u as sigmoid(g) * g * u (Silu itself is
                    # not implemented in CoreSim).
                    sg = spool.tile([P, 512], mybir.dt.float32, tag="sg")
                    nc.scalar.activation(
                        sg[:, :n], psg[ci][:, :n],
                        mybir.ActivationFunctionType.Sigmoid,
                    )
                    nc.vector.tensor_mul(sg[:, :n], sg[:, :n], psg[ci][:, :n])
                    nc.vector.tensor_mul(
                        h_out[:, pr, o:o + n], sg[:, :n], psu[ci][:, :n])

            def g1_pair(s, pr):
                """Routed slot s, SwiGLU pair pr: one packed [g|u] panel
                DMA, 2x16 matmuls, SwiGLU evict into h_t[s]."""
                cap = slot_caps[s]
                pan = wpool.tile([P, KH, 2 * P], bf, tag="wpanel1")
                nc.sync.dma_start(pan[:], w13q[s].ap()[pr])
                psums = []
                for half in range(2):      # 0: g, 1: u
                    ps = [psum1.tile([P, 512], mybir.dt.float32, tag="ps",
                                     name=f"ps_g1_{s}_{pr}_{half}_{ci}")
                          for ci in range(len(_nchunks(cap)))]
                    for k in range(KH):
                        for ci, (o, n) in enumerate(_nchunks(cap)):
                            nc.tensor.matmul(
                                ps[ci][:, :n],
                                lhsT=pan[:, k, half * P:(half + 1) * P],
                                rhs=xg_t[s][:, k, o:o + n],
                                start=(k == 0),
                                stop=(k == KH - 1),
                            )
                    psums.append(ps)
                swiglu(psums[0], psums[1], cap, h_t[s], pr)

            def g1_pair_shared(pr):
                psums = []
                for m in (pr, pr + KS):
                    ps = [psum1.tile([P, 512], mybir.dt.float32, tag="ps",
                                     name=f"ps_g1s_{pr}_{m}_{ci}")
                          for ci in range(len(_nchunks(T)))]
                    for k in range(KH):
                        for ci, (o, n) in enumerate(_nchunks(T)):
                            nc.tensor.matmul(
                                ps[ci][:, :n],
                                lhsT=sgu_t[:, k, m * P:(m + 1) * P],
                                rhs=xt_t[:, k, o:o + n],
                                start=(k == 0),
                                stop=(k == KH - 1),
                            )
                    psums.append(ps)
                swiglu(psums[0], psums[1], T, hs_t, pr)

            def g2_panel(m):
                """GEMM2 m-panel for all routed slots: one merged w2 DMA."""
                pan = w2pool.tile([P, nk_all, P], bf, tag="wpanel2")
                nc.sync.dma_start(pan[:], w2q.ap()[m])
                koff = 0
                for s in range(ns):
                    cap = slot_caps[s]
                    ps = [psum1.tile([P, 512], mybir.dt.float32, tag="ps",
                                     name=f"ps_g2_{m}_{s}_{ci}")
                          for ci in range(len(_nchunks(cap)))]
                    for k in range(nk2s[s]):
                        for ci, (o, n) in enumerate(_nchunks(cap)):
                            nc.tensor.matmul(
                                ps[ci][:, :n],
                                lhsT=pan[:, koff + k, :],
                                rhs=h_t[s][:, k, o:o + n],
                                start=(k == 0),
                                stop=(k == nk2s[s] - 1),
                            )
                    koff += nk2s[s]
                    ot = opool.tile([P, cap], bf, tag=f"yout{s}")
                    for ci, (o, n) in enumerate(_nchunks(cap)):
                        nc.vector.tensor_mul(
                            ot[:, o:o + n], ps[ci][:, :n], wt_t[s][:, o:o + n])
                    nc.sync.dma_start(yout[s].ap()[m], ot[:])

            def sh_g2(m):
                ps = [psum1.tile([P, 512], mybir.dt.float32, tag="ps",
                                 name=f"ps_sh_{m}_{ci}")
                      for ci in range(len(_nchunks(T)))]
                for k in range(KS):
                    for ci, (o, n) in enumerate(_nchunks(T)):
                        nc.tensor.matmul(
                            ps[ci][:, :n],
                            lhsT=sd_t[:, k, m * P:(m + 1) * P],
                            rhs=hs_t[:, k, o:o + n],
                            start=(k == 0),
                            stop=(k == KS - 1),
                        )
                ot = opool.tile([P, T], bf, tag="shout")
                for ci, (o, n) in enumerate(_nchunks(T)):
                    nc.any.tensor_copy(ot[:, o:o + n], ps[ci][:, :n])
                nc.sync.dma_start(shp.ap()[m], ot[:])

            def body():
                # GEMM1: interleave slot pairs so the weight-DMA stream is
                # even; the 3 shared pairs (zero DMA) fill the back half
                # where only the small slot still has pairs.
                np_max = max(npairs)
                shared_at = {np_max - 5: 0, np_max - 3: 1, np_max - 1: 2}
                for pr in range(np_max):
                    for s in range(ns):
                        if pr < npairs[s]:
                            g1_pair(s, pr)
                    j = shared_at.get(pr)
                    if j is not None:
                        g1_pair_shared(j)
                # Shared GEMM2 first: sd is resident (zero DMA), so the PE
                # chews it while the w2 panel stream fills the DMA queue.
                for m in range(MH):
                    sh_g2(m)
                for m in range(MH):
                    g2_panel(m)

            if reps == 1:
                body()
            else:
                with tc.For_i(0, reps, 1):
                    body()

    nc.compile()
    return nc


_PROGRAM_CACHE = {}


def _get_program(CB, CS):
    key = (CB, CS)
    if key not in _PROGRAM_CACHE:
        _PROGRAM_CACHE[key] = _build_program(CB, CS)
    return _PROGRAM_CACHE[key]


TP_A = False        # True: slot A is TP-split (half weights, cap=cmax);
                    # False: slot A is token-split (full weights, cap=cmax/2)


def _assign_slots(counts):
    """Slot-instance assignment. Returns (CA, CB, instA, instB, tail).
    instA: 8 instances -- top-4 experts, TP-split (expert, half) or
    token-split (expert, lo, hi) depending on TP_A.
    instB: 8 instances (expert, lo, hi) -- token-split chunks.
    tail: expert ids computed on host.
    Falls back to (None-tail) big/small token assignment for flat
    distributions."""
    order = np.argsort(-counts, kind="stable")
    rnd8 = lambda v: max(8, int(-(-v // 8)) * 8)
    tail = [int(e) for e in order[10:] if counts[e] > 0]
    tail_tokens = int(sum(counts[e] for e in tail))
    if tail_tokens <= 512:
        instA = []
        if TP_A:
            CA = rnd8(int(counts[order[0]]))
            for i in range(4):
                e = int(order[i])
                instA += [(e, 0), (e, 1)]
        else:
            CA = rnd8(-(-int(counts[order[0]]) // 2))
            for i in range(4):
                e = int(order[i])
                n = int(counts[e])
                n1 = (n + 1) // 2
                instA += [(e, 0, n1), (e, n1, n)]
        mid = [int(order[i]) for i in range(4, 10) if counts[order[i]] > 0]
        CB = rnd8(max(1, int(-(-sum(int(counts[e]) for e in mid) // 8))))
        while sum(-(-int(counts[e]) // CB) for e in mid) > 8:
            CB += 8
        instB = []
        for e in mid:
            n = int(counts[e])
            lo = 0
            while lo < n:
                hi = min(lo + CB, n)
                instB.append((e, lo, hi))
                lo = hi
        while len(instB) < 8:
            instB.append((int(order[0]), 0, 0))   # dead instance
        return CA, CB, instA, instB, tail
    # Fallback: one big + one small expert per core, full experts.
    big, small = order[:N_CORES], order[N_CORES:]
    CA = rnd8(int(counts[big].max()))
    CB = rnd8(int(counts[small].max())) if counts[small].max() > 0 else 0
    instA = [(int(e), 0, int(counts[e])) for e in big]
    instB = [(int(e), 0, int(counts[e])) for e in small[::-1]]
    return CA, CB, instA, instB, None


def _pack_w13_tp(w13_e, half):
    """TP half of w13 for one expert: g rows [j*704:(j+1)*704] and the
    matching u rows, each zero-padded to 768, pair-packed."""
    gu = np.zeros((2 * IH, H), dtype=BF16)
    lo, hi = half * (I // 2), (half + 1) * (I // 2)
    gu[:hi - lo] = w13_e[lo:hi].astype(BF16)
    gu[IH:IH + hi - lo] = w13_e[I + lo:I + hi].astype(BF16)
    return _pack_pairs(gu, NPAIR_TP)


def _pack_w2_tp(w2_e, half):
    """TP half of w2: columns [j*704:(j+1)*704] zero-padded to 768."""
    w2h = np.zeros((H, IH), dtype=BF16)
    lo, hi = half * (I // 2), (half + 1) * (I // 2)
    w2h[:, :hi - lo] = w2_e[:, lo:hi].astype(BF16)
    return _pack_lhs_panels(w2h, MH, IH // P)


def _prepare(x, gate_weight, gate_bias, w13, w2, shared_gate_up, shared_down):
    """Host-side routing + packing. Returns (CB, CS, in_maps, meta)."""
    topk_ids, topk_w = _route(x, gate_weight, gate_bias)
    flat_e = topk_ids.ravel()
    flat_w = topk_w.ravel()
    flat_t = np.repeat(np.arange(T, dtype=np.int64), TOP_K)
    idx_e = [flat_t[flat_e == e] for e in range(E)]
    w_e = [flat_w[flat_e == e] for e in range(E)]
    counts = np.array([len(i) for i in idx_e])

    CB, CS, instA, instB, tail = _assign_slots(counts)
    tp_mode = TP_A and tail is not None
    if tp_mode:
        _SLOT_SPECS[(CB, CS)] = ((NPAIR_TP, NPAIR), (IH // P, KI))
    else:
        ns = 2 if CS else 1
        _SLOT_SPECS[(CB, CS)] = ((NPAIR,) * ns, (KI,) * ns)
        tail = tail if tail is not None else []

    xt_pack = _pack_rhs(x.astype(BF16))                 # [128, KH, T]

    # Pre-pack per-expert full panels once (B instances may share).
    packed13 = {}
    packed2 = {}

    in_maps, meta = [], []
    for c in range(N_CORES):
        insts = [instA[c]] + ([instB[c]] if CS else [])
        caps = [CB] + ([CS] if CS else [])
        im = {}
        cmeta = []
        w2panels = []
        for s, (inst, cap) in enumerate(zip(insts, caps)):
            if tp_mode and s == 0:
                e, half = inst
                idx = idx_e[e]
                im[f"w13q{s}"] = _pack_w13_tp(w13[e], half)
                w2panels.append(_pack_w2_tp(w2[e], half))
            else:
                e, lo, hi = inst
                idx = idx_e[e][lo:hi]
                if e not in packed13:
                    gu = w13[e].astype(BF16)
                    packed13[e] = _pack_pairs(gu, NPAIR)
                    packed2[e] = _pack_lhs_panels(w2[e].astype(BF16), MH, KI)
                im[f"w13q{s}"] = packed13[e]
                w2panels.append(packed2[e])
            n = len(idx)
            xg = np.zeros((cap, H), dtype=BF16)
            xg[:n] = x[idx].astype(BF16)
            im[f"xgq{s}"] = _pack_rhs(xg)
            wt = np.zeros((cap,), dtype=F32)
            wt[:n] = (w_e[inst[0]] if (tp_mode and s == 0)
                      else w_e[inst[0]][inst[1]:inst[2]]).astype(F32)
            im[f"wtb{s}"] = np.ascontiguousarray(
                np.broadcast_to(wt[None, :], (P, cap)).astype(F32))
            cmeta.append((s, inst[0], idx))
        # merged w2 stream: concat along the k-subtile axis
        im["w2q"] = np.ascontiguousarray(np.concatenate(w2panels, axis=2))
        # shared shard: rows [c*352, (c+1)*352) of gate and up, padded to 384
        sh = IS // N_CORES
        lo, hi = c * sh, (c + 1) * sh
        gsl = np.zeros((SSH, H), dtype=F32)
        usl = np.zeros((SSH, H), dtype=F32)
        gsl[:hi - lo] = shared_gate_up[lo:hi]
        usl[:hi - lo] = shared_gate_up[IS + lo:IS + hi]
        sgu_pad = np.concatenate([gsl, usl], 0).astype(BF16)   # [768, H]
        im["sguq"] = _pack_lhs_panels(sgu_pad, 2 * KS, KH)
        sd_sl = np.zeros((H, SSH), dtype=F32)
        sd_sl[:, :hi - lo] = shared_down[:, lo:hi]
        im["sdq"] = _pack_lhs_panels(sd_sl.astype(BF16), MH, KS)
        im["xtq"] = xt_pack
        in_maps.append(im)
        meta.append(cmeta)

    # Tail experts (tiny token counts): computed host-side, f32.
    tail_add = []
    for e in tail:
        idx = idx_e[e]
        if len(idx) == 0:
            continue
        xt32 = x[idx].astype(F32)
        gu = xt32 @ w13[e].astype(F32).T
        y = _silu_mul_np(gu) @ w2[e].astype(F32).T
        tail_add.append((idx, y * w_e[e].astype(F32)[:, None]))
    meta = {"slots": meta, "tail_add": tail_add}
    return CB, CS, in_maps, meta


def _combine(results, meta):
    out = np.zeros((H, T), dtype=F32)
    for c in range(N_CORES):
        out += results[c]["shp"].reshape(H, T).astype(F32)
    out = np.ascontiguousarray(out.T)                   # [T, H]
    for c in range(N_CORES):
        r = results[c]
        for (s, e, idx) in meta["slots"][c]:
            n = len(idx)
            if n:
                y = r[f"y{s}"].reshape(H, -1).astype(F32)   # [H, cap]
                out[idx] += y[:, :n].T
    for idx, y in meta["tail_add"]:
        out[idx] += y
    return out


def kernel(hidden_states, gate_weight, gate_bias, w13, w2,
           shared_gate_up, shared_down):
    x = np.asarray(hidden_states, dtype=F32)
    gate_weight = np.asarray(gate_weight, dtype=F32)
    gate_bias = np.asarray(gate_bias, dtype=F32)
    w13 = np.asarray(w13, dtype=F32)
    w2 = np.asarray(w2, dtype=F32)
    shared_gate_up = np.asarray(shared_gate_up, dtype=F32)
    shared_down = np.asarray(shared_down, dtype=F32)

    CB, CS, in_maps, meta = _prepare(
        x, gate_weight, gate_bias, w13, w2, shared_gate_up, shared_down)
    nc = _get_program(CB, CS)
    res = run_bass_kernel_spmd(nc, in_maps, core_ids=list(range(N_CORES)))
    return _combine(res.results, meta)


# revision 12
# speedup vs baseline: 1.0362x; 1.0112x over previous
"""DeepseekMoE (moe_routing) Trainium2 kernel — round-2 configuration.

Token-split slot assignment (CA=400/CB=176), interleaved GEMM1 pairs and
GEMM2 panels, streamed shared-expert weights, bf16 outputs.
"""

import numpy as np
import ml_dtypes

import concourse.mybir as mybir
import concourse.tile as tile
from concourse import bacc
from concourse.bass_utils import run_bass_kernel_spmd

BF16 = ml_dtypes.bfloat16
F32 = np.float32

T, H, E, I = 1024, 2048, 16, 1408
I2 = 2 * I
IS = 2 * I
SSH = 384
TOP_K, N_GROUP, TOPK_GROUP = 4, 4, 2
ROUTED_SCALE = 2.5
N_CORES = 8
P = 128
KH = H // P
KI = I // P
MW = I2 // P
MH = H // P
NPAIR = I // P
KS = SSH // P


def _sigmoid(x):
    return 1.0 / (1.0 + np.exp(-x))


def _route(x, gate_weight, gate_bias):
    logits = x.astype(np.float64) @ gate_weight.astype(np.float64).T
    scores = _sigmoid(logits)
    choice = scores + gate_bias.astype(np.float64)[None, :]
    g = choice.reshape(T, N_GROUP, E // N_GROUP)
    top2sum = np.sort(g, axis=-1)[..., -2:].sum(-1)
    gidx = np.argsort(-top2sum, axis=-1, kind="stable")[:, :TOPK_GROUP]
    gmask = np.zeros((T, N_GROUP), bool)
    gmask[np.arange(T)[:, None], gidx] = True
    emask = np.repeat(gmask, E // N_GROUP, axis=1)
    masked = np.where(emask, choice, -np.inf)
    topk_ids = np.argsort(-masked, axis=-1, kind="stable")[:, :TOP_K]
    topk_w = np.take_along_axis(scores, topk_ids, axis=1)
    topk_w = topk_w / topk_w.sum(-1, keepdims=True) * ROUTED_SCALE
    return topk_ids.astype(np.int32), topk_w


def _silu_mul_np(gu):
    g, u = gu[:, :gu.shape[1] // 2], gu[:, gu.shape[1] // 2:]
    return (g / (1.0 + np.exp(-g))) * u


def _pack_lhs_panels(w, n_m, n_k):
    a = w.reshape(n_m, P, n_k, P)
    return np.ascontiguousarray(a.transpose(0, 3, 2, 1))


def _pack_rhs(xcols):
    a = xcols.reshape(-1, KH, P)
    return np.ascontiguousarray(a.transpose(2, 1, 0))


def _nchunks(c):
    out = []
    o = 0
    while o < c:
        n = min(512, c - o)
        out.append((o, n))
        o += n
    return out


def _build_program(CB, CS, reps=1):
    nc = bacc.Bacc(None, target_bir_lowering=False)
    bf = mybir.dt.bfloat16
    f32 = mybir.dt.float32

    slot_caps = [c for c in (CB, CS) if c > 0]
    ns = len(slot_caps)

    w13q = [nc.dram_tensor(f"w13q{s}", [MW, P, KH, P], bf, kind="ExternalInput")
            for s in range(ns)]
    w2q = [nc.dram_tensor(f"w2q{s}", [MH, P, KI, P], bf, kind="ExternalInput")
           for s in range(ns)]
    xgq = [nc.dram_tensor(f"xgq{s}", [P, KH, slot_caps[s]], bf, kind="ExternalInput")
           for s in range(ns)]
    wtb = [nc.dram_tensor(f"wtb{s}", [P, slot_caps[s]], f32, kind="ExternalInput")
           for s in range(ns)]
    sguq = nc.dram_tensor("sguq", [2 * KS, P, KH, P], bf, kind="ExternalInput")
    sdq = nc.dram_tensor("sdq", [MH, P, KS, P], bf, kind="ExternalInput")
    xtq = nc.dram_tensor("xtq", [P, KH, T], bf, kind="ExternalInput")

    yout = [nc.dram_tensor(f"y{s}", [MH, P, slot_caps[s]], bf, kind="ExternalOutput")
            for s in range(ns)]
    shp = nc.dram_tensor("shp", [MH, P, T], bf, kind="ExternalOutput")

    with tile.TileContext(nc) as tc:
        with (
            tc.tile_pool(name="resident", bufs=1) as res,
            tc.tile_pool(name="wpanel", bufs=8) as wpool,
            tc.tile_pool(name="hbuf", bufs=1) as hpool,
            tc.tile_pool(name="silu", bufs=4) as spool,
            tc.tile_pool(name="outbuf", bufs=4) as opool,
            tc.tile_pool(name="psum", bufs=8, space="PSUM") as psum1,
        ):
            xg_t, wt_t = [], []
            for s in range(ns):
                c = slot_caps[s]
                t = res.tile([P, KH, c], bf, name=f"xg{s}_t")
                nc.sync.dma_start(t[:], xgq[s].ap()[:])
                xg_t.append(t)
                w = res.tile([P, c], f32, name=f"wt{s}_t")
                nc.sync.dma_start(w[:], wtb[s].ap()[:])
                wt_t.append(w)
            xt_t = res.tile([P, KH, T], bf)
            nc.sync.dma_start(xt_t[:], xtq.ap()[:])
            sd_t = res.tile([P, KS, H], bf)
            for m in range(MH):
                nc.sync.dma_start(sd_t[:, :, m * P:(m + 1) * P], sdq.ap()[m])

            h_t = [hpool.tile([P, KI, slot_caps[s]], bf, name=f"h{s}_t", tag=f"h{s}_t")
                   for s in range(ns)]
            hs_t = hpool.tile([P, KS, T], bf)

            def g1_pair(wq_ap, rhs_t, pr, pair_gap, n_k, cap, h_out):
                panels, psums = [], []
                for m in (pr, pr + pair_gap):
                    pan = wpool.tile([P, KH, P], bf, tag="wpanel1")
                    nc.sync.dma_start(pan[:, :n_k, :], wq_ap[m])
                    panels.append(pan)
                    ps = [psum1.tile([P, 512], mybir.dt.float32, tag="ps",
                                     name=f"ps_g1_{pr}_{m}_{ci}")
                          for ci in range(len(_nchunks(cap)))]
                    for k in range(n_k):
                        for ci, (o, n) in enumerate(_nchunks(cap)):
                            nc.tensor.matmul(
                                ps[ci][:, :n],
                                lhsT=pan[:, k, :],
                                rhs=rhs_t[:, k, o:o + n],
                                start=(k == 0),
                                stop=(k == n_k - 1),
                            )
                    psums.append(ps)
                for ci, (o, n) in enumerate(_nchunks(cap)):
                    sg = spool.tile([P, 512], mybir.dt.float32, tag="sg")
                    nc.scalar.activation(
                        sg[:, :n], psums[0][ci][:, :n],
                        mybir.ActivationFunctionType.Sigmoid,
                    )
                    nc.vector.tensor_mul(sg[:, :n], sg[:, :n], psums[0][ci][:, :n])
                    nc.vector.tensor_mul(
                        h_out[:, pr, o:o + n], sg[:, :n], psums[1][ci][:, :n])

            def g2_panel(wq_ap, h_in, m, n_k, cap, out_dram, scale_t):
                pan = wpool.tile([P, KI, P], bf, tag="wpanel2")
                nc.sync.dma_start(pan[:, :n_k, :], wq_ap[m])
                ps = [psum1.tile([P, 512], mybir.dt.float32, tag="ps",
                                 name=f"ps_g2_{m}_{ci}")
                      for ci in range(len(_nchunks(cap)))]
                for k in range(n_k):
                    for ci, (o, n) in enumerate(_nchunks(cap)):
                        nc.tensor.matmul(
                            ps[ci][:, :n],
                            lhsT=pan[:, k, :],
                            rhs=h_in[:, k, o:o + n],
                            start=(k == 0),
                            stop=(k == n_k - 1),
                        )
                ot = opool.tile([P, cap], bf, tag="yout")
                for ci, (o, n) in enumerate(_nchunks(cap)):
                    nc.vector.tensor_mul(
                        ot[:, o:o + n], ps[ci][:, :n], scale_t[:, o:o + n])
                nc.sync.dma_start(out_dram.ap()[m], ot[:])

            def sh_g2(m):
                ps = [psum1.tile([P, 512], mybir.dt.float32, tag="ps",
                                 name=f"ps_sh_{m}_{ci}")
                      for ci in range(len(_nchunks(T)))]
                for k in range(KS):
                    for ci, (o, n) in enumerate(_nchunks(T)):
                        nc.tensor.matmul(
                            ps[ci][:, :n],
                            lhsT=sd_t[:, k, m * P:(m + 1) * P],
                            rhs=hs_t[:, k, o:o + n],
                            start=(k == 0),
                            stop=(k == KS - 1),
                        )
                ot = opool.tile([P, T], bf, tag="shout")
                for ci, (o, n) in enumerate(_nchunks(T)):
                    nc.any.tensor_copy(ot[:, o:o + n], ps[ci][:, :n])
                nc.sync.dma_start(shp.ap()[m], ot[:])

            def body():
                for pr in range(NPAIR):
                    for s in range(ns):
                        g1_pair(w13q[s].ap(), xg_t[s], pr, NPAIR, KH,
                                slot_caps[s], h_t[s])
                    if pr >= 2 and (pr - 2) % 3 == 0 and (pr - 2) // 3 < KS:
                        g1_pair(sguq.ap(), xt_t, (pr - 2) // 3, KS, KH, T,
                                hs_t)
                for m in range(MH):
                    sh_g2(m)
                for m in range(MH):
                    for s in range(ns):
                        g2_panel(w2q[s].ap(), h_t[s], m, KI, slot_caps[s],
                                 yout[s], wt_t[s])

            if reps == 1:
                body()
            else:
                with tc.For_i(0, reps, 1):
                    body()

    nc.compile()
    return nc


_PROGRAM_CACHE = {}


def _get_program(CB, CS):
    key = (CB, CS)
    if key not in _PROGRAM_CACHE:
        _PROGRAM_CACHE[key] = _build_program(CB, CS)
    return _PROGRAM_CACHE[key]


def _assign_slots(counts):
    order = np.argsort(-counts, kind="stable")
    rnd8 = lambda v: max(8, int(-(-v // 8)) * 8)
    tail = [int(e) for e in order[10:] if counts[e] > 0]
    tail_tokens = int(sum(counts[e] for e in tail))
    if tail_tokens <= 512:
        CA = rnd8(-(-int(counts[order[0]]) // 2))
        instA = []
        for i in range(4):
            e = int(order[i])
            n = int(counts[e])
            n1 = (n + 1) // 2
            instA += [(e, 0, n1), (e, n1, n)]
        mid = [int(order[i]) for i in range(4, 10) if counts[order[i]] > 0]
        CB = rnd8(max(1, int(-(-sum(int(counts[e]) for e in mid) // 8))))
        while sum(-(-int(counts[e]) // CB) for e in mid) > 8:
            CB += 8
        instB = []
        for e in mid:
            n = int(counts[e])
            lo = 0
            while lo < n:
                hi = min(lo + CB, n)
                instB.append((e, lo, hi))
                lo = hi
        while len(instB) < 8:
            instB.append((int(order[0]), 0, 0))
        return CA, CB, instA, instB, tail
    big, small = order[:N_CORES], order[N_CORES:]
    CA = rnd8(int(counts[big].max()))
    CB = rnd8(int(counts[small].max())) if counts[small].max() > 0 else 0
    instA = [(int(e), 0, int(counts[e])) for e in big]
    instB = [(int(e), 0, int(counts[e])) for e in small[::-1]]
    return CA, CB, instA, instB, []


def _prepare(x, gate_weight, gate_bias, w13, w2, shared_gate_up, shared_down):
    topk_ids, topk_w = _route(x, gate_weight, gate_bias)
    flat_e = topk_ids.ravel()
    flat_w = topk_w.ravel()
    flat_t = np.repeat(np.arange(T, dtype=np.int64), TOP_K)
    idx_e = [flat_t[flat_e == e] for e in range(E)]
    w_e = [flat_w[flat_e == e] for e in range(E)]
    counts = np.array([len(i) for i in idx_e])

    CB, CS, instA, instB, tail = _assign_slots(counts)

    xt_pack = _pack_rhs(x.astype(BF16))

    packed13, packed2 = {}, {}
    in_maps, meta = [], []
    for c in range(N_CORES):
        insts = [instA[c]] + ([instB[c]] if CS else [])
        caps = [CB] + ([CS] if CS else [])
        im = {}
        cmeta = []
        for s, ((e, lo, hi), cap) in enumerate(zip(insts, caps)):
            idx = idx_e[e][lo:hi]
            n = len(idx)
            xg = np.zeros((cap, H), dtype=BF16)
            xg[:n] = x[idx].astype(BF16)
            im[f"xgq{s}"] = _pack_rhs(xg)
            wt = np.zeros((cap,), dtype=F32)
            wt[:n] = w_e[e][lo:hi].astype(F32)
            im[f"wtb{s}"] = np.ascontiguousarray(
                np.broadcast_to(wt[None, :], (P, cap)).astype(F32))
            if e not in packed13:
                packed13[e] = _pack_lhs_panels(w13[e].astype(BF16), MW, KH)
                packed2[e] = _pack_lhs_panels(w2[e].astype(BF16), MH, KI)
            im[f"w13q{s}"] = packed13[e]
            im[f"w2q{s}"] = packed2[e]
            cmeta.append((s, e, idx))
        sh = IS // N_CORES
        lo, hi = c * sh, (c + 1) * sh
        gsl = np.zeros((SSH, H), dtype=F32)
        usl = np.zeros((SSH, H), dtype=F32)
        gsl[:hi - lo] = shared_gate_up[lo:hi]
        usl[:hi - lo] = shared_gate_up[IS + lo:IS + hi]
        sgu_pad = np.concatenate([gsl, usl], 0).astype(BF16)
        im["sguq"] = _pack_lhs_panels(sgu_pad, 2 * KS, KH)
        sd_sl = np.zeros((H, SSH), dtype=F32)
        sd_sl[:, :hi - lo] = shared_down[:, lo:hi]
        im["sdq"] = _pack_lhs_panels(sd_sl.astype(BF16), MH, KS)
        im["xtq"] = xt_pack
        in_maps.append(im)
        meta.append(cmeta)

    tail_add = []
    for e in tail:
        idx = idx_e[e]
        if len(idx) == 0:
            continue
        xt32 = x[idx].astype(F32)
        gu = xt32 @ w13[e].astype(F32).T
        y = _silu_mul_np(gu) @ w2[e].astype(F32).T
        tail_add.append((idx, y * w_e[e].astype(F32)[:, None]))
    meta = {"slots": meta, "tail_add": tail_add}
    return CB, CS, in_maps, meta


def _combine(results, meta):
    out = np.zeros((H, T), dtype=F32)
    for c in range(N_CORES):
        out += results[c]["shp"].reshape(H, T).astype(F32)
    out = np.ascontiguousarray(out.T)
    for c in range(N_CORES):
        r = results[c]
        for (s, e, idx) in meta["slots"][c]:
            n = len(idx)
            if n:
                y = r[f"y{s}"].reshape(H, -1).astype(F32)
                out[idx] += y[:, :n].T
    for idx, y in meta["tail_add"]:
        out[idx] += y
    return out


def kernel(hidden_states, gate_weight, gate_bias, w13, w2,
           shared_gate_up, shared_down):
    x = np.asarray(hidden_states, dtype=F32)
    gate_weight = np.asarray(gate_weight, dtype=F32)
    gate_bias = np.asarray(gate_bias, dtype=F32)
    w13 = np.asarray(w13, dtype=F32)
    w2 = np.asarray(w2, dtype=F32)
    shared_gate_up = np.asarray(shared_gate_up, dtype=F32)
    shared_down = np.asarray(shared_down, dtype=F32)

    CB, CS, in_maps, meta = _prepare(
        x, gate_weight, gate_bias, w13, w2, shared_gate_up, shared_down)
    nc = _get_program(CB, CS)
    res = run_bass_kernel_spmd(nc, in_maps, core_ids=list(range(N_CORES)))
    return _combine(res.results, meta)


# revision 14
# speedup vs baseline: 1.0730x; 1.0355x over previous
"""DeepseekMoE (moe_routing) Trainium2 kernel — round-2 configuration.

Token-split slot assignment (CA=400/CB=176), interleaved GEMM1 pairs and
GEMM2 panels, streamed shared-expert weights, bf16 outputs.
"""

import numpy as np
import ml_dtypes

import concourse.mybir as mybir
import concourse.tile as tile
from concourse import bacc
from concourse.bass_utils import run_bass_kernel_spmd

BF16 = ml_dtypes.bfloat16
F32 = np.float32

T, H, E, I = 1024, 2048, 16, 1408
I2 = 2 * I
IS = 2 * I
SSH = 384
TOP_K, N_GROUP, TOPK_GROUP = 4, 4, 2
ROUTED_SCALE = 2.5
N_CORES = 8
P = 128
KH = H // P
KI = I // P
MW = I2 // P
MH = H // P
NPAIR = I // P
KS = SSH // P


def _sigmoid(x):
    return 1.0 / (1.0 + np.exp(-x))


def _route(x, gate_weight, gate_bias):
    logits = x.astype(np.float64) @ gate_weight.astype(np.float64).T
    scores = _sigmoid(logits)
    choice = scores + gate_bias.astype(np.float64)[None, :]
    g = choice.reshape(T, N_GROUP, E // N_GROUP)
    top2sum = np.sort(g, axis=-1)[..., -2:].sum(-1)
    gidx = np.argsort(-top2sum, axis=-1, kind="stable")[:, :TOPK_GROUP]
    gmask = np.zeros((T, N_GROUP), bool)
    gmask[np.arange(T)[:, None], gidx] = True
    emask = np.repeat(gmask, E // N_GROUP, axis=1)
    masked = np.where(emask, choice, -np.inf)
    topk_ids = np.argsort(-masked, axis=-1, kind="stable")[:, :TOP_K]
    topk_w = np.take_along_axis(scores, topk_ids, axis=1)
    topk_w = topk_w / topk_w.sum(-1, keepdims=True) * ROUTED_SCALE
    return topk_ids.astype(np.int32), topk_w


def _silu_mul_np(gu):
    g, u = gu[:, :gu.shape[1] // 2], gu[:, gu.shape[1] // 2:]
    return (g / (1.0 + np.exp(-g))) * u


def _pack_lhs_panels(w, n_m, n_k):
    a = w.reshape(n_m, P, n_k, P)
    return np.ascontiguousarray(a.transpose(0, 3, 2, 1))


def _pack_rhs(xcols):
    a = xcols.reshape(-1, KH, P)
    return np.ascontiguousarray(a.transpose(2, 1, 0))


def _nchunks(c):
    out = []
    o = 0
    while o < c:
        n = min(512, c - o)
        out.append((o, n))
        o += n
    return out


def _build_program(CB, CS, reps=1):
    nc = bacc.Bacc(None, target_bir_lowering=False)
    bf = mybir.dt.bfloat16
    f32 = mybir.dt.float32

    slot_caps = [c for c in (CB, CS) if c > 0]
    ns = len(slot_caps)

    w13q = [nc.dram_tensor(f"w13q{s}", [MW, P, KH, P], bf, kind="ExternalInput")
            for s in range(ns)]
    w2q = [nc.dram_tensor(f"w2q{s}", [MH, P, KI, P], bf, kind="ExternalInput")
           for s in range(ns)]
    xgq = [nc.dram_tensor(f"xgq{s}", [P, KH, slot_caps[s]], bf, kind="ExternalInput")
           for s in range(ns)]
    wtb = [nc.dram_tensor(f"wtb{s}", [P, slot_caps[s]], f32, kind="ExternalInput")
           for s in range(ns)]
    sguq = nc.dram_tensor("sguq", [2 * KS, P, KH, P], bf, kind="ExternalInput")
    sdq = nc.dram_tensor("sdq", [MH, P, KS, P], bf, kind="ExternalInput")
    xtq = nc.dram_tensor("xtq", [P, KH, T], bf, kind="ExternalInput")

    yout = [nc.dram_tensor(f"y{s}", [MH, P, slot_caps[s]], bf, kind="ExternalOutput")
            for s in range(ns)]
    shp = nc.dram_tensor("shp", [MH, P, T], bf, kind="ExternalOutput")

    with tile.TileContext(nc) as tc:
        with (
            tc.tile_pool(name="resident", bufs=1) as res,
            tc.tile_pool(name="wpanel", bufs=8) as wpool,
            tc.tile_pool(name="hbuf", bufs=1) as hpool,
            tc.tile_pool(name="silu", bufs=4) as spool,
            tc.tile_pool(name="outbuf", bufs=4) as opool,
            tc.tile_pool(name="psum", bufs=8, space="PSUM") as psum1,
        ):
            xg_t, wt_t = [], []
            for s in range(ns):
                c = slot_caps[s]
                t = res.tile([P, KH, c], bf, name=f"xg{s}_t")
                nc.sync.dma_start(t[:], xgq[s].ap()[:])
                xg_t.append(t)
                w = res.tile([P, c], f32, name=f"wt{s}_t")
                nc.sync.dma_start(w[:], wtb[s].ap()[:])
                wt_t.append(w)
            xt_t = res.tile([P, KH, T], bf)
            nc.sync.dma_start(xt_t[:], xtq.ap()[:])
            sd_t = res.tile([P, KS, H], bf)
            for m in range(MH):
                nc.sync.dma_start(sd_t[:, :, m * P:(m + 1) * P], sdq.ap()[m])

            h_t = [hpool.tile([P, KI, slot_caps[s]], bf, name=f"h{s}_t", tag=f"h{s}_t")
                   for s in range(ns)]
            hs_t = hpool.tile([P, KS, T], bf)

            def g1_pair(wq_ap, rhs_t, pr, pair_gap, n_k, cap, h_out):
                panels, psums = [], []
                for m in (pr, pr + pair_gap):
                    pan = wpool.tile([P, KH, P], bf, tag="wpanel1")
                    nc.sync.dma_start(pan[:, :n_k, :], wq_ap[m])
                    panels.append(pan)
                    ps = [psum1.tile([P, 512], mybir.dt.float32, tag="ps",
                                     name=f"ps_g1_{pr}_{m}_{ci}")
                          for ci in range(len(_nchunks(cap)))]
                    for k in range(n_k):
                        for ci, (o, n) in enumerate(_nchunks(cap)):
                            nc.tensor.matmul(
                                ps[ci][:, :n],
                                lhsT=pan[:, k, :],
                                rhs=rhs_t[:, k, o:o + n],
                                start=(k == 0),
                                stop=(k == n_k - 1),
                            )
                    psums.append(ps)
                for ci, (o, n) in enumerate(_nchunks(cap)):
                    sg = spool.tile([P, 512], mybir.dt.float32, tag="sg")
                    nc.scalar.activation(
                        sg[:, :n], psums[0][ci][:, :n],
                        mybir.ActivationFunctionType.Sigmoid,
                    )
                    nc.vector.tensor_mul(sg[:, :n], sg[:, :n], psums[0][ci][:, :n])
                    nc.vector.tensor_mul(
                        h_out[:, pr, o:o + n], sg[:, :n], psums[1][ci][:, :n])

            def g2_panel(wq_ap, h_in, m, n_k, cap, out_dram, scale_t):
                pan = wpool.tile([P, KI, P], bf, tag="wpanel2")
                nc.sync.dma_start(pan[:, :n_k, :], wq_ap[m])
                ps = [psum1.tile([P, 512], mybir.dt.float32, tag="ps",
                                 name=f"ps_g2_{m}_{ci}")
                      for ci in range(len(_nchunks(cap)))]
                for k in range(n_k):
                    for ci, (o, n) in enumerate(_nchunks(cap)):
                        nc.tensor.matmul(
                            ps[ci][:, :n],
                            lhsT=pan[:, k, :],
                            rhs=h_in[:, k, o:o + n],
                            start=(k == 0),
                            stop=(k == n_k - 1),
                        )
                ot = opool.tile([P, cap], bf, tag="yout")
                for ci, (o, n) in enumerate(_nchunks(cap)):
                    nc.vector.tensor_mul(
                        ot[:, o:o + n], ps[ci][:, :n], scale_t[:, o:o + n])
                nc.sync.dma_start(out_dram.ap()[m], ot[:])

            def sh_g2(m):
                ps = [psum1.tile([P, 512], mybir.dt.float32, tag="ps",
                                 name=f"ps_sh_{m}_{ci}")
                      for ci in range(len(_nchunks(T)))]
                for k in range(KS):
                    for ci, (o, n) in enumerate(_nchunks(T)):
                        nc.tensor.matmul(
                            ps[ci][:, :n],
                            lhsT=sd_t[:, k, m * P:(m + 1) * P],
                            rhs=hs_t[:, k, o:o + n],
                            start=(k == 0),
                            stop=(k == KS - 1),
                        )
                ot = opool.tile([P, T], bf, tag="shout")
                for ci, (o, n) in enumerate(_nchunks(T)):
                    nc.any.tensor_copy(ot[:, o:o + n], ps[ci][:, :n])
                nc.sync.dma_start(shp.ap()[m], ot[:])

            def body():
                for pr in range(NPAIR):
                    for s in range(ns):
                        g1_pair(w13q[s].ap(), xg_t[s], pr, NPAIR, KH,
                                slot_caps[s], h_t[s])
                    if pr >= 2 and (pr - 2) % 3 == 0 and (pr - 2) // 3 < KS:
                        g1_pair(sguq.ap(), xt_t, (pr - 2) // 3, KS, KH, T,
                                hs_t)
                for m in range(MH):
                    sh_g2(m)
                for m in range(MH):
                    for s in range(ns):
                        g2_panel(w2q[s].ap(), h_t[s], m, KI, slot_caps[s],
                                 yout[s], wt_t[s])

            if reps == 1:
                body()
            else:
                with tc.For_i(0, reps, 1):
                    body()

    nc.compile()
    return nc


_PROGRAM_CACHE = {}


def _get_program(CB, CS):
    key = (CB, CS)
    if key not in _PROGRAM_CACHE:
        _PROGRAM_CACHE[key] = _build_program(CB, CS)
    return _PROGRAM_CACHE[key]


def _assign_slots(counts):
    order = np.argsort(-counts, kind="stable")
    rnd8 = lambda v: max(8, int(-(-v // 8)) * 8)
    tail = [int(e) for e in order[10:] if counts[e] > 0]
    tail_tokens = int(sum(counts[e] for e in tail))
    if tail_tokens <= 512:
        CA = rnd8(-(-int(counts[order[0]]) // 2))
        instA = []
        for i in range(4):
            e = int(order[i])
            n = int(counts[e])
            n1 = (n + 1) // 2
            instA += [(e, 0, n1), (e, n1, n)]
        mid = [int(order[i]) for i in range(4, 10) if counts[order[i]] > 0]
        CB = rnd8(max(1, int(-(-sum(int(counts[e]) for e in mid) // 8))))
        while sum(-(-int(counts[e]) // CB) for e in mid) > 8:
            CB += 8
        instB = []
        for e in mid:
            n = int(counts[e])
            lo = 0
            while lo < n:
                hi = min(lo + CB, n)
                instB.append((e, lo, hi))
                lo = hi
        while len(instB) < 8:
            instB.append((int(order[0]), 0, 0))
        return CA, CB, instA, instB, tail
    big, small = order[:N_CORES], order[N_CORES:]
    CA = rnd8(int(counts[big].max()))
    CB = rnd8(int(counts[small].max())) if counts[small].max() > 0 else 0
    instA = [(int(e), 0, int(counts[e])) for e in big]
    instB = [(int(e), 0, int(counts[e])) for e in small[::-1]]
    return CA, CB, instA, instB, []


def _prepare(x, gate_weight, gate_bias, w13, w2, shared_gate_up, shared_down):
    topk_ids, topk_w = _route(x, gate_weight, gate_bias)
    flat_e = topk_ids.ravel()
    flat_w = topk_w.ravel()
    flat_t = np.repeat(np.arange(T, dtype=np.int64), TOP_K)
    idx_e = [flat_t[flat_e == e] for e in range(E)]
    w_e = [flat_w[flat_e == e] for e in range(E)]
    counts = np.array([len(i) for i in idx_e])

    CB, CS, instA, instB, tail = _assign_slots(counts)

    xt_pack = _pack_rhs(x.astype(BF16))

    packed13, packed2 = {}, {}
    in_maps, meta = [], []
    for c in range(N_CORES):
        insts = [instA[c]] + ([instB[c]] if CS else [])
        caps = [CB] + ([CS] if CS else [])
        im = {}
        cmeta = []
        for s, ((e, lo, hi), cap) in enumerate(zip(insts, caps)):
            idx = idx_e[e][lo:hi]
            n = len(idx)
            xg = np.zeros((cap, H), dtype=BF16)
            xg[:n] = x[idx].astype(BF16)
            im[f"xgq{s}"] = _pack_rhs(xg)
            wt = np.zeros((cap,), dtype=F32)
            wt[:n] = w_e[e][lo:hi].astype(F32)
            im[f"wtb{s}"] = np.ascontiguousarray(
                np.broadcast_to(wt[None, :], (P, cap)).astype(F32))
            if e not in packed13:
                packed13[e] = _pack_lhs_panels(w13[e].astype(BF16), MW, KH)
                packed2[e] = _pack_lhs_panels(w2[e].astype(BF16), MH, KI)
            im[f"w13q{s}"] = packed13[e]
            im[f"w2q{s}"] = packed2[e]
            cmeta.append((s, e, idx))
        sh = IS // N_CORES
        lo, hi = c * sh, (c + 1) * sh
        gsl = np.zeros((SSH, H), dtype=F32)
        usl = np.zeros((SSH, H), dtype=F32)
        gsl[:hi - lo] = shared_gate_up[lo:hi]
        usl[:hi - lo] = shared_gate_up[IS + lo:IS + hi]
        sgu_pad = np.concatenate([gsl, usl], 0).astype(BF16)
        im["sguq"] = _pack_lhs_panels(sgu_pad, 2 * KS, KH)
        sd_sl = np.zeros((H, SSH), dtype=F32)
        sd_sl[:, :hi - lo] = shared_down[:, lo:hi]
        im["sdq"] = _pack_lhs_panels(sd_sl.astype(BF16), MH, KS)
        im["xtq"] = xt_pack
        in_maps.append(im)
        meta.append(cmeta)

    tail_add = []
    for e in tail:
        idx = idx_e[e]
        if len(idx) == 0:
            continue
        xt32 = x[idx].astype(F32)
        gu = xt32 @ w13[e].astype(F32).T
        y = _silu_mul_np(gu) @ w2[e].astype(F32).T
        tail_add.append((idx, y * w_e[e].astype(F32)[:, None]))
    meta = {"slots": meta, "tail_add": tail_add}
    return CB, CS, in_maps, meta


def _combine(results, meta):
    out = np.zeros((H, T), dtype=F32)
    for c in range(N_CORES):
        out += results[c]["shp"].reshape(H, T).astype(F32)
    out = np.ascontiguousarray(out.T)
    for c in range(N_CORES):
        r = results[c]
        for (s, e, idx) in meta["slots"][c]:
            n = len(idx)
            if n:
                y = r[f"y{s}"].reshape(H, -1).astype(F32)
                out[idx] += y[:, :n].T
    for idx, y in meta["tail_add"]:
        out[idx] += y
    return out


def kernel(hidden_states, gate_weight, gate_bias, w13, w2,
           shared_gate_up, shared_down):
    x = np.asarray(hidden_states, dtype=F32)
    gate_weight = np.asarray(gate_weight, dtype=F32)
    gate_bias = np.asarray(gate_bias, dtype=F32)
    w13 = np.asarray(w13, dtype=F32)
    w2 = np.asarray(w2, dtype=F32)
    shared_gate_up = np.asarray(shared_gate_up, dtype=F32)
    shared_down = np.asarray(shared_down, dtype=F32)

    CB, CS, in_maps, meta = _prepare(
        x, gate_weight, gate_bias, w13, w2, shared_gate_up, shared_down)
    nc = _get_program(CB, CS)
    res = run_bass_kernel_spmd(nc, in_maps, core_ids=list(range(N_CORES)))
    return _combine(res.results, meta)
